# revision 4
# baseline (speedup 1.0000x reference)
"""Distributed causal multi-head attention for Trainium2 (8 NeuronCores).

Problem: B=2, S=2048, d_model=1024, 16 heads x 64 dims, causal softmax attention.

Tensor-parallel over heads (2 heads/core), partial-sum output projection
(each core computes its 128-feature slice of the Wo contraction over all
rows; host sums the 8 partial outputs).

Schedule:
  - X^T in 8 per-kc sbuf tiles (separate tiles keep DMA region tracking
    exact); columns loaded range-major (512/512/1024/2048) with issues
    split across the GpSimd and Sync queues so arrival matches rc-major
    consumption; the wq weight slab is DMA'd first so the first projection
    chain starts ~6us after the preamble.
  - Per chunk: q-chain, k-chain, scores(0,1), v-chain, V-transposes, then
    the k-tile loop with depth-2 score prefetch — the early score tiles
    start Scalar's exp pipeline ~2.5us sooner than chains-first order.
  - Output projection of chunk rc-1 drip-fed one unit (512-col matmul +
    psum->sbuf copy) per k-tile inside chunk rc's attention loop; copies
    on Vector (Scalar helps during low-exp chunks and the tail);
    back-loaded in the final chunk so PE has work while the last exps
    drain.
  - One output DMA per (b,qc) chunk via a rearranged AP on Sync; per-rt
    DMAs in the tail so the last chunk drains incrementally.
  - V-natural tiles built with one 3D-AP copy + one strided memset.
"""
import os
import sys

sys.path.insert(0, "/opt/trn_rl_repo")

import numpy as np
import ml_dtypes

from concourse import bacc, mybir, tile
from concourse.bass_utils import run_bass_kernel_spmd

BF16 = mybir.dt.bfloat16
F32 = mybir.dt.float32

B, S, DM = 2, 2048, 1024
H, DK = 16, 64
N_CORES = 8
FPC = 128           # features per core = 2 heads x 64
NKT = S // 128      # k-tiles per batch = 16
NQC = S // 512      # q-chunks per batch = 4
SCALE = 1.0 / 8.0   # 1/sqrt(64)

_cache = {}


def _build():
    nc = bacc.Bacc("TRN2", target_bir_lowering=False, debug=False, num_devices=N_CORES)

    xt = nc.dram_tensor("xt", [DM, B * S], BF16, kind="ExternalInput")
    # wpk[p, :]: [wq|wk|wv tiles (3*8*128)] + [wo (1024)] + [mask|ident (256)]
    wpk = nc.dram_tensor("wpk", [128, 3 * 8 * 128 + DM + 256], BF16, kind="ExternalInput")
    bpk = nc.dram_tensor("bpk", [FPC, 3], F32, kind="ExternalInput")
    out_ext = nc.dram_tensor("out", [B, S, DM], BF16, kind="ExternalOutput")

    EXP = mybir.ActivationFunctionType.Exp
    IDENT = mybir.ActivationFunctionType.Identity

    with tile.TileContext(nc) as tc:
        with (
            tc.tile_pool(name="xtp", bufs=1) as xtp,
            tc.tile_pool(name="wts", bufs=1) as wts,
            tc.tile_pool(name="qkv", bufs=1) as qkvp,
            tc.tile_pool(name="vnat", bufs=1) as vnatp,
            tc.tile_pool(name="work", bufs=3) as work,
            tc.tile_pool(name="stage", bufs=2) as stagep,
            tc.tile_pool(name="outp", bufs=2) as outp,
            tc.tile_pool(name="psmm", bufs=2, space="PSUM") as psmm,
            tc.tile_pool(name="psS", bufs=2, space="PSUM") as psS,
            tc.tile_pool(name="psO", bufs=1, space="PSUM") as psO,
        ):
            # ---------- weights + X^T loads ----------
            WPK_N = 3 * 8 * 128 + DM + 256
            wpk_sb = wts.tile([128, WPK_N], BF16, tag="wpk", name="wpk_sb")
            nc.gpsimd.dma_start(wpk_sb[:, 0:1024], wpk[:, 0:1024])

            xt_sb = []
            for kc in range(8):
                t = xtp.tile([128, B * S], BF16, tag=f"xt{kc}", name=f"xt{kc}")
                xt_sb.append(t)
            # first 512 cols of every kc, split across GpSimd and Sync queues
            for kc in range(0, 8, 2):
                nc.gpsimd.dma_start(
                    xt_sb[kc][:, 0:512], xt[kc * 128:(kc + 1) * 128, 0:512]
                )
                nc.sync.dma_start(
                    xt_sb[kc + 1][:, 0:512], xt[(kc + 1) * 128:(kc + 2) * 128, 0:512]
                )
            nc.gpsimd.dma_start(wpk_sb[:, 1024:3072], wpk[:, 1024:3072])
            nc.gpsimd.dma_start(wpk_sb[:, 3072:WPK_N], wpk[:, 3072:WPK_N])
            bpk_sb = wts.tile([FPC, 3], F32, tag="bpk", name="bpk_sb")
            nc.gpsimd.dma_start(bpk_sb[:], bpk[:])
            # remaining columns chunk-major: [512:1024] split across both
            # queues, the rest on Sync
            for kc in range(0, 8, 2):
                nc.sync.dma_start(
                    xt_sb[kc][:, 512:1024], xt[kc * 128:(kc + 1) * 128, 512:1024]
                )
                nc.gpsimd.dma_start(
                    xt_sb[kc + 1][:, 512:1024], xt[(kc + 1) * 128:(kc + 2) * 128, 512:1024]
                )
            for c0, c1 in ((1024, 2048), (2048, 4096)):
                for kc in range(8):
                    nc.sync.dma_start(
                        xt_sb[kc][:, c0:c1], xt[kc * 128:(kc + 1) * 128, c0:c1]
                    )

            def wslice(pr, kc):
                o = (pr * 8 + kc) * 128
                return wpk_sb[:, o:o + 128]

            wq_sb = [wslice(0, kc) for kc in range(8)]
            wk_sb = [wslice(1, kc) for kc in range(8)]
            wv_sb = [wslice(2, kc) for kc in range(8)]
            wo_sb = wpk_sb[:, 3072:3072 + DM]
            mask_sb = wpk_sb[:, 3072 + DM:3072 + DM + 128]
            ident_sb = wpk_sb[:, 3072 + DM + 128:3072 + DM + 256]
            b_sb = {"q": bpk_sb[:, 0:1], "k": bpk_sb[:, 1:2], "v": bpk_sb[:, 2:3]}

            # ---------- fully interleaved main loop ----------
            proj_sb = {}
            for name in ("q", "k", "v"):
                proj_sb[name] = qkvp.tile(
                    [128, B * S], BF16, tag=f"{name}T", name=f"{name}T"
                )
            qT, kT, vT = proj_sb["q"], proj_sb["k"], proj_sb["v"]
            w_by_name = {"q": wq_sb, "k": wk_sb, "v": wv_sb}
            v_nat = [[None] * NKT for _ in range(B)]

            # outproj work queue: each item emits one 512-col matmul + copy;
            # the chunk's DMA fires after its 8th unit.
            unit_q = []

            def emit_unit(scalar_ok=False, fine_dma=False):
                b, qc, rt, nc_i, ot, o_all = unit_q.pop(0)
                ps = psmm.tile([128, 512], F32, tag="mm",
                               name=f"pso{b}_{qc}_{rt}_{nc_i}")
                nc.tensor.matmul(
                    ps[:], ot[:, rt * 128:(rt + 1) * 128],
                    wo_sb[:, nc_i * 512:(nc_i + 1) * 512],
                    start=True, stop=True,
                )
                dst = o_all[:, rt * 1024 + nc_i * 512: rt * 1024 + (nc_i + 1) * 512]
                if scalar_ok and nc_i == 1:
                    nc.scalar.copy(dst, ps[:])
                else:
                    nc.vector.tensor_copy(dst, ps[:])
                if fine_dma and nc_i == 1:
                    nc.sync.dma_start(
                        out_ext[b, qc * 512 + rt * 128: qc * 512 + (rt + 1) * 128, :],
                        o_all[:, rt * 1024:(rt + 1) * 1024],
                    )
                elif rt == 3 and nc_i == 1:
                    dmadst = out_ext[b, qc * 512:(qc + 1) * 512, :].rearrange(
                        "(rt p) d -> p rt d", rt=4, p=128
                    )
                    nc.sync.dma_start(
                        dmadst, o_all[:].rearrange("p (rt d) -> p rt d", rt=4)
                    )

            def queue_outproj(b, qc, ot):
                o_all = outp.tile([128, 4096], BF16, tag="oall", name=f"oall{b}_{qc}")
                for rt in range(4):
                    for nc_i in range(2):
                        unit_q.append((b, qc, rt, nc_i, ot, o_all))

            for rc in range(8):
                b, qc = (0, rc) if rc < 4 else (1, rc - 4)
                nkt = 4 * qc + 4
                o_ps = [
                    psO.tile([65, 512], F32, tag=f"o{h}", name=f"o_ps{h}_{b}_{qc}")
                    for h in (0, 1)
                ]

                def emit_s(kt):
                    d = 128 * (kt - 4 * qc)
                    lo = max(0, d)
                    k_sl = slice(b * S + kt * 128, b * S + (kt + 1) * 128)
                    s_ps = psS.tile([128, 1024], F32, tag="s", name=f"s_{b}_{qc}_{kt}")
                    q_lo = slice(b * S + qc * 512 + lo, b * S + (qc + 1) * 512)
                    for h in (0, 1):
                        hp = slice(64 * h, 64 * h + 64)
                        nc.tensor.matmul(
                            s_ps[:, 512 * h + lo:512 * h + 512],
                            kT[hp, k_sl], qT[hp, q_lo],
                            start=True, stop=True,
                        )
                    return s_ps, lo, d

                def emit_chain(name):
                    ps = psmm.tile([128, 512], F32, tag="mm", name=f"ps_{name}{rc}")
                    for kc in range(8):
                        nc.tensor.matmul(
                            ps[:], w_by_name[name][kc],
                            xt_sb[kc][:, rc * 512:(rc + 1) * 512],
                            start=(kc == 0), stop=(kc == 7),
                        )
                    nc.scalar.activation(
                        proj_sb[name][:, rc * 512:(rc + 1) * 512], ps[:], IDENT,
                        bias=b_sb[name],
                    )

                emit_chain("q")
                emit_chain("k")
                # first two score tiles right away so Scalar's exp pipeline
                # starts before the v-chain and transposes
                s_cur = emit_s(0)
                s_nxt0 = emit_s(1) if nkt > 1 else None
                emit_chain("v")
                for kt in range(4 * qc, 4 * qc + 4):
                    ps = psmm.tile([128, 128], BF16, tag="mm", name=f"pst{b}_{kt}")
                    nc.tensor.transpose(
                        ps[:], vT[:, b * S + kt * 128: b * S + (kt + 1) * 128],
                        ident_sb,
                    )
                    vn = vnatp.tile([128, 130], BF16, tag=f"vn{b}_{kt}", name=f"vn{b}_{kt}")
                    nc.vector.tensor_copy(
                        vn[:, 0:130].rearrange("p (g c) -> p g c", g=2, c=65)[:, :, 0:64],
                        ps[:].rearrange("p (g c) -> p g c", g=2, c=64),
                    )
                    nc.vector.memset(
                        vn[:, 0:130].rearrange("p (g c) -> p g c", g=2, c=65)[:, :, 64:65],
                        1.0,
                    )
                    v_nat[b][kt] = vn
                last_chunk = rc == 7
                if last_chunk:
                    upk = 1
                    unit_start = nkt - len(unit_q)
                else:
                    upk = -(-len(unit_q) // nkt) if unit_q else 0  # units per kt
                    unit_start = 0
                scalar_ok = nkt <= 8  # scalar helps copies in low-exp chunks

                for kt in range(nkt):
                    s_ps, lo, d = s_cur
                    if kt + 2 < nkt:
                        s_nxt = s_nxt0
                        s_nxt0 = emit_s(kt + 2)
                    elif kt + 1 < nkt:
                        s_nxt = s_nxt0
                        s_nxt0 = None
                    else:
                        s_nxt = None
                    p_sb = work.tile([128, 1024], BF16, tag="p", name=f"p_{b}_{qc}_{kt}")
                    nc.scalar.activation(
                        p_sb[:, lo:1024], s_ps[:, lo:1024], EXP, scale=SCALE,
                    )
                    if d >= 0:
                        hi = min(512, d + 128)
                        for h in (0, 1):
                            nc.vector.tensor_mul(
                                p_sb[:, 512 * h + lo:512 * h + hi],
                                p_sb[:, 512 * h + lo:512 * h + hi],
                                mask_sb[:, 0:hi - lo],
                            )
                    for h in (0, 1):
                        nc.tensor.matmul(
                            o_ps[h][:, lo:512],
                            v_nat[b][kt][:, 65 * h:65 * h + 65],
                            p_sb[:, 512 * h + lo:512 * h + 512],
                            start=(kt == 0), stop=(kt == nkt - 1),
                        )
                    if kt >= unit_start:
                        for _ in range(min(upk, len(unit_q))):
                            emit_unit(scalar_ok=scalar_ok)
                    s_cur = s_nxt
                # normalize (per-head denominator on psum row 64) + stage
                ot = stagep.tile([128, 512], BF16, tag="ot", name=f"ot{b}_{qc}")
                for h in (0, 1):
                    rc_sb = work.tile([128, 512], F32, tag="recip", name=f"rc{b}_{qc}_{h}")
                    nc.vector.tensor_copy(rc_sb[0:1, :], o_ps[h][64:65, :])
                    nc.vector.reciprocal_approx_fast(rc_sb[0:1, :], rc_sb[0:1, :])
                    nc.gpsimd.partition_broadcast(
                        rc_sb[0:64, :], rc_sb[0:1, :], channels=64
                    )
                    nc.vector.tensor_mul(
                        ot[64 * h:64 * h + 64, :], o_ps[h][0:64, :], rc_sb[0:64, :]
                    )
                queue_outproj(b, qc, ot)

            while unit_q:
                emit_unit(scalar_ok=True, fine_dma=True)

    nc.compile()
    return nc


def kernel(x, Wq, bq, Wk, bk, Wv, bv, Wo):
    if "nc" not in _cache:
        _cache["nc"] = _build()
    nc = _cache["nc"]

    bf = ml_dtypes.bfloat16
    xt = np.ascontiguousarray(np.asarray(x, np.float32).reshape(B * S, DM).T).astype(bf)
    wo_f = np.asarray(Wo, np.float32)
    trimask = np.triu(np.ones((128, 128), np.float32))
    ident = np.eye(128, dtype=np.float32)

    in_maps = []
    for c in range(N_CORES):
        sl = slice(c * FPC, (c + 1) * FPC)
        wpk = np.empty((128, 3 * 8 * 128 + DM + 256), np.float32)
        for pr, W in enumerate((Wq, Wk, Wv)):
            Wc = np.asarray(W, np.float32)[:, sl]          # [1024, 128]
            wpk[:, pr * 1024:(pr + 1) * 1024] = (
                Wc.reshape(8, 128, 128).transpose(1, 0, 2).reshape(128, 1024)
            )
        wpk[:, 3072:3072 + DM] = wo_f[sl, :]
        wpk[:, 3072 + DM:3072 + DM + 128] = trimask
        wpk[:, 3072 + DM + 128:] = ident
        bpk = np.stack(
            [np.asarray(b, np.float32)[sl] for b in (bq, bk, bv)], axis=1
        )
        in_maps.append({
            "xt": xt,
            "wpk": np.ascontiguousarray(wpk).astype(bf),
            "bpk": np.ascontiguousarray(bpk),
        })

    trace = bool(int(os.environ.get("ATTN_KERNEL_TRACE", "0")))
    res = run_bass_kernel_spmd(nc, in_maps, core_ids=list(range(N_CORES)), trace=trace)
    if trace:
        print(f"HW exec time: {res.exec_time_ns} ns")
        _cache["exec_time_ns"] = res.exec_time_ns

    out = np.asarray(res.results[0]["out"]).astype(np.float32)
    for c in range(1, N_CORES):
        out += np.asarray(res.results[c]["out"]).astype(np.float32)
    return out


# revision 5
# speedup vs baseline: 1.0097x; 1.0097x over previous
"""Distributed causal multi-head attention for Trainium2 (8 NeuronCores).

Problem: B=2, S=2048, d_model=1024, 16 heads x 64 dims, causal softmax attention.

Tensor-parallel over heads (2 heads/core), partial-sum output projection
(each core computes its 128-feature slice of the Wo contraction over all
rows; host sums the 8 partial outputs).

Schedule:
  - X^T in 8 per-kc sbuf tiles (separate tiles keep DMA region tracking
    exact); columns loaded range-major (512/512/1024/2048) with issues
    split across the GpSimd and Sync queues so arrival matches rc-major
    consumption; the wq weight slab is DMA'd first so the first projection
    chain starts ~6us after the preamble.
  - Per chunk: q-chain, k-chain, scores(0,1), v-chain, V-transposes, then
    the k-tile loop with depth-2 score prefetch — the early score tiles
    start Scalar's exp pipeline ~2.5us sooner than chains-first order.
  - Output projection drip-fed at a flat one unit (512-col matmul +
    psum->sbuf copy) per k-tile inside later chunks' attention loops,
    leftovers rolling forward; copies on Vector (Scalar helps during
    low-exp chunks and the tail);
    back-loaded in the final chunk so PE has work while the last exps
    drain.
  - One output DMA per (b,qc) chunk via a rearranged AP on Sync; per-rt
    DMAs in the tail so the last chunk drains incrementally.
  - V-natural tiles built with one 3D-AP copy + one strided memset.
"""
import os
import sys

sys.path.insert(0, "/opt/trn_rl_repo")

import numpy as np
import ml_dtypes

from concourse import bacc, mybir, tile
from concourse.bass_utils import run_bass_kernel_spmd

BF16 = mybir.dt.bfloat16
F32 = mybir.dt.float32

B, S, DM = 2, 2048, 1024
H, DK = 16, 64
N_CORES = 8
FPC = 128           # features per core = 2 heads x 64
NKT = S // 128      # k-tiles per batch = 16
NQC = S // 512      # q-chunks per batch = 4
SCALE = 1.0 / 8.0   # 1/sqrt(64)

_cache = {}


def _build():
    nc = bacc.Bacc("TRN2", target_bir_lowering=False, debug=False, num_devices=N_CORES)

    xt = nc.dram_tensor("xt", [DM, B * S], BF16, kind="ExternalInput")
    # wpk[p, :]: [wq|wk|wv tiles (3*8*128)] + [wo (1024)] + [mask|ident (256)]
    wpk = nc.dram_tensor("wpk", [128, 3 * 8 * 128 + DM + 256], BF16, kind="ExternalInput")
    bpk = nc.dram_tensor("bpk", [FPC, 3], F32, kind="ExternalInput")
    out_ext = nc.dram_tensor("out", [B, S, DM], BF16, kind="ExternalOutput")

    EXP = mybir.ActivationFunctionType.Exp
    IDENT = mybir.ActivationFunctionType.Identity

    with tile.TileContext(nc) as tc:
        with (
            tc.tile_pool(name="xtp", bufs=1) as xtp,
            tc.tile_pool(name="wts", bufs=1) as wts,
            tc.tile_pool(name="qkv", bufs=1) as qkvp,
            tc.tile_pool(name="vnat", bufs=1) as vnatp,
            tc.tile_pool(name="work", bufs=3) as work,
            tc.tile_pool(name="stage", bufs=2) as stagep,
            tc.tile_pool(name="outp", bufs=3) as outp,
            tc.tile_pool(name="psmm", bufs=2, space="PSUM") as psmm,
            tc.tile_pool(name="psS", bufs=2, space="PSUM") as psS,
            tc.tile_pool(name="psO", bufs=1, space="PSUM") as psO,
        ):
            # ---------- weights + X^T loads ----------
            WPK_N = 3 * 8 * 128 + DM + 256
            wpk_sb = wts.tile([128, WPK_N], BF16, tag="wpk", name="wpk_sb")
            nc.gpsimd.dma_start(wpk_sb[:, 0:1024], wpk[:, 0:1024])

            xt_sb = []
            for kc in range(8):
                t = xtp.tile([128, B * S], BF16, tag=f"xt{kc}", name=f"xt{kc}")
                xt_sb.append(t)
            # first 512 cols of every kc, split across GpSimd and Sync queues
            for kc in range(0, 8, 2):
                nc.gpsimd.dma_start(
                    xt_sb[kc][:, 0:512], xt[kc * 128:(kc + 1) * 128, 0:512]
                )
                nc.sync.dma_start(
                    xt_sb[kc + 1][:, 0:512], xt[(kc + 1) * 128:(kc + 2) * 128, 0:512]
                )
            nc.gpsimd.dma_start(wpk_sb[:, 1024:3072], wpk[:, 1024:3072])
            nc.gpsimd.dma_start(wpk_sb[:, 3072:WPK_N], wpk[:, 3072:WPK_N])
            bpk_sb = wts.tile([FPC, 3], F32, tag="bpk", name="bpk_sb")
            nc.gpsimd.dma_start(bpk_sb[:], bpk[:])
            # remaining columns chunk-major: [512:1024] split across both
            # queues, the rest on Sync
            for kc in range(0, 8, 2):
                nc.sync.dma_start(
                    xt_sb[kc][:, 512:1024], xt[kc * 128:(kc + 1) * 128, 512:1024]
                )
                nc.gpsimd.dma_start(
                    xt_sb[kc + 1][:, 512:1024], xt[(kc + 1) * 128:(kc + 2) * 128, 512:1024]
                )
            for c0, c1 in ((1024, 2048), (2048, 4096)):
                for kc in range(8):
                    nc.sync.dma_start(
                        xt_sb[kc][:, c0:c1], xt[kc * 128:(kc + 1) * 128, c0:c1]
                    )

            def wslice(pr, kc):
                o = (pr * 8 + kc) * 128
                return wpk_sb[:, o:o + 128]

            wq_sb = [wslice(0, kc) for kc in range(8)]
            wk_sb = [wslice(1, kc) for kc in range(8)]
            wv_sb = [wslice(2, kc) for kc in range(8)]
            wo_sb = wpk_sb[:, 3072:3072 + DM]
            mask_sb = wpk_sb[:, 3072 + DM:3072 + DM + 128]
            ident_sb = wpk_sb[:, 3072 + DM + 128:3072 + DM + 256]
            b_sb = {"q": bpk_sb[:, 0:1], "k": bpk_sb[:, 1:2], "v": bpk_sb[:, 2:3]}

            # ---------- fully interleaved main loop ----------
            proj_sb = {}
            for name in ("q", "k", "v"):
                proj_sb[name] = qkvp.tile(
                    [128, B * S], BF16, tag=f"{name}T", name=f"{name}T"
                )
            qT, kT, vT = proj_sb["q"], proj_sb["k"], proj_sb["v"]
            w_by_name = {"q": wq_sb, "k": wk_sb, "v": wv_sb}
            v_nat = [[None] * NKT for _ in range(B)]

            # outproj work queue: each item emits one 512-col matmul + copy;
            # the chunk's DMA fires after its 8th unit.
            unit_q = []

            def emit_unit(scalar_ok=False, fine_dma=False):
                b, qc, rt, nc_i, ot, o_all = unit_q.pop(0)
                ps = psmm.tile([128, 512], F32, tag="mm",
                               name=f"pso{b}_{qc}_{rt}_{nc_i}")
                nc.tensor.matmul(
                    ps[:], ot[:, rt * 128:(rt + 1) * 128],
                    wo_sb[:, nc_i * 512:(nc_i + 1) * 512],
                    start=True, stop=True,
                )
                dst = o_all[:, rt * 1024 + nc_i * 512: rt * 1024 + (nc_i + 1) * 512]
                if scalar_ok and nc_i == 1:
                    nc.scalar.copy(dst, ps[:])
                else:
                    nc.vector.tensor_copy(dst, ps[:])
                if fine_dma and nc_i == 1:
                    nc.sync.dma_start(
                        out_ext[b, qc * 512 + rt * 128: qc * 512 + (rt + 1) * 128, :],
                        o_all[:, rt * 1024:(rt + 1) * 1024],
                    )
                elif rt == 3 and nc_i == 1:
                    dmadst = out_ext[b, qc * 512:(qc + 1) * 512, :].rearrange(
                        "(rt p) d -> p rt d", rt=4, p=128
                    )
                    nc.sync.dma_start(
                        dmadst, o_all[:].rearrange("p (rt d) -> p rt d", rt=4)
                    )

            def queue_outproj(b, qc, ot):
                o_all = outp.tile([128, 4096], BF16, tag="oall", name=f"oall{b}_{qc}")
                for rt in range(4):
                    for nc_i in range(2):
                        unit_q.append((b, qc, rt, nc_i, ot, o_all))

            for rc in range(8):
                b, qc = (0, rc) if rc < 4 else (1, rc - 4)
                nkt = 4 * qc + 4
                o_ps = [
                    psO.tile([65, 512], F32, tag=f"o{h}", name=f"o_ps{h}_{b}_{qc}")
                    for h in (0, 1)
                ]

                def emit_s(kt):
                    d = 128 * (kt - 4 * qc)
                    lo = max(0, d)
                    k_sl = slice(b * S + kt * 128, b * S + (kt + 1) * 128)
                    s_ps = psS.tile([128, 1024], F32, tag="s", name=f"s_{b}_{qc}_{kt}")
                    q_lo = slice(b * S + qc * 512 + lo, b * S + (qc + 1) * 512)
                    for h in (0, 1):
                        hp = slice(64 * h, 64 * h + 64)
                        nc.tensor.matmul(
                            s_ps[:, 512 * h + lo:512 * h + 512],
                            kT[hp, k_sl], qT[hp, q_lo],
                            start=True, stop=True,
                        )
                    return s_ps, lo, d

                def emit_chain(name):
                    ps = psmm.tile([128, 512], F32, tag="mm", name=f"ps_{name}{rc}")
                    for kc in range(8):
                        nc.tensor.matmul(
                            ps[:], w_by_name[name][kc],
                            xt_sb[kc][:, rc * 512:(rc + 1) * 512],
                            start=(kc == 0), stop=(kc == 7),
                        )
                    nc.scalar.activation(
                        proj_sb[name][:, rc * 512:(rc + 1) * 512], ps[:], IDENT,
                        bias=b_sb[name],
                    )

                emit_chain("q")
                emit_chain("k")
                # first two score tiles right away so Scalar's exp pipeline
                # starts before the v-chain and transposes
                s_cur = emit_s(0)
                s_nxt0 = emit_s(1) if nkt > 1 else None
                emit_chain("v")
                for kt in range(4 * qc, 4 * qc + 4):
                    ps = psmm.tile([128, 128], BF16, tag="mm", name=f"pst{b}_{kt}")
                    nc.tensor.transpose(
                        ps[:], vT[:, b * S + kt * 128: b * S + (kt + 1) * 128],
                        ident_sb,
                    )
                    vn = vnatp.tile([128, 130], BF16, tag=f"vn{b}_{kt}", name=f"vn{b}_{kt}")
                    nc.vector.tensor_copy(
                        vn[:, 0:130].rearrange("p (g c) -> p g c", g=2, c=65)[:, :, 0:64],
                        ps[:].rearrange("p (g c) -> p g c", g=2, c=64),
                    )
                    nc.vector.memset(
                        vn[:, 0:130].rearrange("p (g c) -> p g c", g=2, c=65)[:, :, 64:65],
                        1.0,
                    )
                    v_nat[b][kt] = vn
                last_chunk = rc == 7
                if last_chunk:
                    upk = 1
                    unit_start = max(0, nkt - len(unit_q))
                else:
                    upk = 1 if unit_q else 0  # flat pacing; leftovers roll over
                    unit_start = 0
                scalar_ok = nkt <= 8  # scalar helps copies in low-exp chunks

                for kt in range(nkt):
                    s_ps, lo, d = s_cur
                    if kt + 2 < nkt:
                        s_nxt = s_nxt0
                        s_nxt0 = emit_s(kt + 2)
                    elif kt + 1 < nkt:
                        s_nxt = s_nxt0
                        s_nxt0 = None
                    else:
                        s_nxt = None
                    p_sb = work.tile([128, 1024], BF16, tag="p", name=f"p_{b}_{qc}_{kt}")
                    nc.scalar.activation(
                        p_sb[:, lo:1024], s_ps[:, lo:1024], EXP, scale=SCALE,
                    )
                    if d >= 0:
                        hi = min(512, d + 128)
                        for h in (0, 1):
                            nc.vector.tensor_mul(
                                p_sb[:, 512 * h + lo:512 * h + hi],
                                p_sb[:, 512 * h + lo:512 * h + hi],
                                mask_sb[:, 0:hi - lo],
                            )
                    for h in (0, 1):
                        nc.tensor.matmul(
                            o_ps[h][:, lo:512],
                            v_nat[b][kt][:, 65 * h:65 * h + 65],
                            p_sb[:, 512 * h + lo:512 * h + 512],
                            start=(kt == 0), stop=(kt == nkt - 1),
                        )
                    if kt >= unit_start:
                        for _ in range(min(upk, len(unit_q))):
                            emit_unit(scalar_ok=scalar_ok)
                    s_cur = s_nxt
                # normalize (per-head denominator on psum row 64) + stage
                ot = stagep.tile([128, 512], BF16, tag="ot", name=f"ot{b}_{qc}")
                for h in (0, 1):
                    rc_sb = work.tile([128, 512], F32, tag="recip", name=f"rc{b}_{qc}_{h}")
                    nc.vector.tensor_copy(rc_sb[0:1, :], o_ps[h][64:65, :])
                    nc.vector.reciprocal_approx_fast(rc_sb[0:1, :], rc_sb[0:1, :])
                    nc.gpsimd.partition_broadcast(
                        rc_sb[0:64, :], rc_sb[0:1, :], channels=64
                    )
                    nc.vector.tensor_mul(
                        ot[64 * h:64 * h + 64, :], o_ps[h][0:64, :], rc_sb[0:64, :]
                    )
                queue_outproj(b, qc, ot)

            while unit_q:
                emit_unit(scalar_ok=True, fine_dma=True)

    nc.compile()
    return nc


def kernel(x, Wq, bq, Wk, bk, Wv, bv, Wo):
    if "nc" not in _cache:
        _cache["nc"] = _build()
    nc = _cache["nc"]

    bf = ml_dtypes.bfloat16
    xt = np.ascontiguousarray(np.asarray(x, np.float32).reshape(B * S, DM).T).astype(bf)
    wo_f = np.asarray(Wo, np.float32)
    trimask = np.triu(np.ones((128, 128), np.float32))
    ident = np.eye(128, dtype=np.float32)

    in_maps = []
    for c in range(N_CORES):
        sl = slice(c * FPC, (c + 1) * FPC)
        wpk = np.empty((128, 3 * 8 * 128 + DM + 256), np.float32)
        for pr, W in enumerate((Wq, Wk, Wv)):
            Wc = np.asarray(W, np.float32)[:, sl]          # [1024, 128]
            wpk[:, pr * 1024:(pr + 1) * 1024] = (
                Wc.reshape(8, 128, 128).transpose(1, 0, 2).reshape(128, 1024)
            )
        wpk[:, 3072:3072 + DM] = wo_f[sl, :]
        wpk[:, 3072 + DM:3072 + DM + 128] = trimask
        wpk[:, 3072 + DM + 128:] = ident
        bpk = np.stack(
            [np.asarray(b, np.float32)[sl] for b in (bq, bk, bv)], axis=1
        )
        in_maps.append({
            "xt": xt,
            "wpk": np.ascontiguousarray(wpk).astype(bf),
            "bpk": np.ascontiguousarray(bpk),
        })

    trace = bool(int(os.environ.get("ATTN_KERNEL_TRACE", "0")))
    res = run_bass_kernel_spmd(nc, in_maps, core_ids=list(range(N_CORES)), trace=trace)
    if trace:
        print(f"HW exec time: {res.exec_time_ns} ns")
        _cache["exec_time_ns"] = res.exec_time_ns

    out = np.asarray(res.results[0]["out"]).astype(np.float32)
    for c in range(1, N_CORES):
        out += np.asarray(res.results[c]["out"]).astype(np.float32)
    return out


# revision 6
# speedup vs baseline: 1.0184x; 1.0086x over previous
"""Distributed causal multi-head attention for Trainium2 (8 NeuronCores).

Problem: B=2, S=2048, d_model=1024, 16 heads x 64 dims, causal softmax attention.

Tensor-parallel over heads (2 heads/core), partial-sum output projection
(each core computes its 128-feature slice of the Wo contraction over all
rows; host sums the 8 partial outputs).

Schedule:
  - X^T in 8 per-kc sbuf tiles (separate tiles keep DMA region tracking
    exact); columns loaded range-major (512/512/1024/2048) with issues
    split across the GpSimd and Sync queues so arrival matches rc-major
    consumption; the wq weight slab is DMA'd first so the first projection
    chain starts ~6us after the preamble.
  - Per chunk: q-chain, k-chain, scores(0,1), v-chain, V-transposes, then
    the k-tile loop with depth-2 score prefetch — the early score tiles
    start Scalar's exp pipeline ~2.5us sooner than chains-first order.
  - Output projection drip-fed at a flat one unit (512-col matmul +
    psum->sbuf copy) per k-tile inside later chunks' attention loops,
    leftovers rolling forward; copies on Vector (Scalar helps during
    low-exp chunks and the tail);
    back-loaded in the final chunk so PE has work while the last exps
    drain.
  - One output DMA per (b,qc) chunk via a rearranged AP on Sync; per-rt
    DMAs in the tail so the last chunk drains incrementally.
  - V-natural tiles built with one 3D-AP copy + one strided memset.
"""
import os
import sys

sys.path.insert(0, "/opt/trn_rl_repo")

import numpy as np
import ml_dtypes

from concourse import bacc, mybir, tile
from concourse.bass_utils import run_bass_kernel_spmd

BF16 = mybir.dt.bfloat16
F32 = mybir.dt.float32

B, S, DM = 2, 2048, 1024
H, DK = 16, 64
N_CORES = 8
FPC = 128           # features per core = 2 heads x 64
NKT = S // 128      # k-tiles per batch = 16
NQC = S // 512      # q-chunks per batch = 4
SCALE = 1.0 / 8.0   # 1/sqrt(64)

_cache = {}


def _build():
    nc = bacc.Bacc("TRN2", target_bir_lowering=False, debug=False, num_devices=N_CORES)

    xt = nc.dram_tensor("xt", [DM, B * S], BF16, kind="ExternalInput")
    # wpk[p, :]: [wq|wk|wv tiles (3*8*128)] + [wo (1024)] + [mask|ident (256)]
    wpk = nc.dram_tensor("wpk", [128, 3 * 8 * 128 + DM + 256], BF16, kind="ExternalInput")
    bpk = nc.dram_tensor("bpk", [FPC, 3], F32, kind="ExternalInput")
    out_ext = nc.dram_tensor("out", [B, S, DM], BF16, kind="ExternalOutput")

    EXP = mybir.ActivationFunctionType.Exp
    IDENT = mybir.ActivationFunctionType.Identity

    with tile.TileContext(nc) as tc:
        with (
            tc.tile_pool(name="xtp", bufs=1) as xtp,
            tc.tile_pool(name="wts", bufs=1) as wts,
            tc.tile_pool(name="qkv", bufs=1) as qkvp,
            tc.tile_pool(name="vnat", bufs=1) as vnatp,
            tc.tile_pool(name="work", bufs=4) as work,
            tc.tile_pool(name="stage", bufs=3) as stagep,
            tc.tile_pool(name="outp", bufs=3) as outp,
            tc.tile_pool(name="psmm", bufs=2, space="PSUM") as psmm,
            tc.tile_pool(name="psS", bufs=2, space="PSUM") as psS,
            tc.tile_pool(name="psO", bufs=1, space="PSUM") as psO,
        ):
            # ---------- weights + X^T loads ----------
            WPK_N = 3 * 8 * 128 + DM + 256
            wpk_sb = wts.tile([128, WPK_N], BF16, tag="wpk", name="wpk_sb")
            nc.gpsimd.dma_start(wpk_sb[:, 0:1024], wpk[:, 0:1024])

            xt_sb = []
            for kc in range(8):
                t = xtp.tile([128, B * S], BF16, tag=f"xt{kc}", name=f"xt{kc}")
                xt_sb.append(t)
            # first 512 cols of every kc, split across GpSimd and Sync queues
            for kc in range(0, 8, 2):
                nc.gpsimd.dma_start(
                    xt_sb[kc][:, 0:512], xt[kc * 128:(kc + 1) * 128, 0:512]
                )
                nc.sync.dma_start(
                    xt_sb[kc + 1][:, 0:512], xt[(kc + 1) * 128:(kc + 2) * 128, 0:512]
                )
            nc.gpsimd.dma_start(wpk_sb[:, 1024:3072], wpk[:, 1024:3072])
            nc.gpsimd.dma_start(wpk_sb[:, 3072:WPK_N], wpk[:, 3072:WPK_N])
            bpk_sb = wts.tile([FPC, 3], F32, tag="bpk", name="bpk_sb")
            nc.gpsimd.dma_start(bpk_sb[:], bpk[:])
            # remaining columns chunk-major: [512:1024] split across both
            # queues, the rest on Sync
            for kc in range(0, 8, 2):
                nc.sync.dma_start(
                    xt_sb[kc][:, 512:1024], xt[kc * 128:(kc + 1) * 128, 512:1024]
                )
                nc.gpsimd.dma_start(
                    xt_sb[kc + 1][:, 512:1024], xt[(kc + 1) * 128:(kc + 2) * 128, 512:1024]
                )
            for c0, c1 in ((1024, 2048), (2048, 4096)):
                for kc in range(8):
                    nc.sync.dma_start(
                        xt_sb[kc][:, c0:c1], xt[kc * 128:(kc + 1) * 128, c0:c1]
                    )

            def wslice(pr, kc):
                o = (pr * 8 + kc) * 128
                return wpk_sb[:, o:o + 128]

            wq_sb = [wslice(0, kc) for kc in range(8)]
            wk_sb = [wslice(1, kc) for kc in range(8)]
            wv_sb = [wslice(2, kc) for kc in range(8)]
            wo_sb = wpk_sb[:, 3072:3072 + DM]
            mask_sb = wpk_sb[:, 3072 + DM:3072 + DM + 128]
            ident_sb = wpk_sb[:, 3072 + DM + 128:3072 + DM + 256]
            b_sb = {"q": bpk_sb[:, 0:1], "k": bpk_sb[:, 1:2], "v": bpk_sb[:, 2:3]}

            # ---------- fully interleaved main loop ----------
            proj_sb = {}
            for name in ("q", "k", "v"):
                proj_sb[name] = qkvp.tile(
                    [128, B * S], BF16, tag=f"{name}T", name=f"{name}T"
                )
            qT, kT, vT = proj_sb["q"], proj_sb["k"], proj_sb["v"]
            w_by_name = {"q": wq_sb, "k": wk_sb, "v": wv_sb}
            v_nat = [[None] * NKT for _ in range(B)]

            # outproj work queue: each item emits one 512-col matmul + copy;
            # the chunk's DMA fires after its 8th unit.
            unit_q = []

            def emit_unit(scalar_ok=False, fine_dma=False):
                b, qc, rt, nc_i, ot, o_all = unit_q.pop(0)
                ps = psmm.tile([128, 512], F32, tag="mm",
                               name=f"pso{b}_{qc}_{rt}_{nc_i}")
                nc.tensor.matmul(
                    ps[:], ot[:, rt * 128:(rt + 1) * 128],
                    wo_sb[:, nc_i * 512:(nc_i + 1) * 512],
                    start=True, stop=True,
                )
                dst = o_all[:, rt * 1024 + nc_i * 512: rt * 1024 + (nc_i + 1) * 512]
                if scalar_ok and nc_i == 1:
                    nc.scalar.copy(dst, ps[:])
                else:
                    nc.vector.tensor_copy(dst, ps[:])
                if fine_dma and nc_i == 1:
                    nc.sync.dma_start(
                        out_ext[b, qc * 512 + rt * 128: qc * 512 + (rt + 1) * 128, :],
                        o_all[:, rt * 1024:(rt + 1) * 1024],
                    )
                elif rt == 3 and nc_i == 1:
                    dmadst = out_ext[b, qc * 512:(qc + 1) * 512, :].rearrange(
                        "(rt p) d -> p rt d", rt=4, p=128
                    )
                    nc.sync.dma_start(
                        dmadst, o_all[:].rearrange("p (rt d) -> p rt d", rt=4)
                    )

            def queue_outproj(b, qc, ot):
                o_all = outp.tile([128, 4096], BF16, tag="oall", name=f"oall{b}_{qc}")
                for rt in range(4):
                    for nc_i in range(2):
                        unit_q.append((b, qc, rt, nc_i, ot, o_all))

            for rc in range(8):
                b, qc = (0, rc) if rc < 4 else (1, rc - 4)
                nkt = 4 * qc + 4
                o_ps = [
                    psO.tile([65, 512], F32, tag=f"o{h}", name=f"o_ps{h}_{b}_{qc}")
                    for h in (0, 1)
                ]

                def emit_s(kt):
                    d = 128 * (kt - 4 * qc)
                    lo = max(0, d)
                    k_sl = slice(b * S + kt * 128, b * S + (kt + 1) * 128)
                    s_ps = psS.tile([128, 1024], F32, tag="s", name=f"s_{b}_{qc}_{kt}")
                    q_lo = slice(b * S + qc * 512 + lo, b * S + (qc + 1) * 512)
                    for h in (0, 1):
                        hp = slice(64 * h, 64 * h + 64)
                        nc.tensor.matmul(
                            s_ps[:, 512 * h + lo:512 * h + 512],
                            kT[hp, k_sl], qT[hp, q_lo],
                            start=True, stop=True,
                        )
                    return s_ps, lo, d

                def emit_chain(name):
                    ps = psmm.tile([128, 512], F32, tag="mm", name=f"ps_{name}{rc}")
                    for kc in range(8):
                        nc.tensor.matmul(
                            ps[:], w_by_name[name][kc],
                            xt_sb[kc][:, rc * 512:(rc + 1) * 512],
                            start=(kc == 0), stop=(kc == 7),
                        )
                    nc.scalar.activation(
                        proj_sb[name][:, rc * 512:(rc + 1) * 512], ps[:], IDENT,
                        bias=b_sb[name],
                    )

                emit_chain("q")
                emit_chain("k")
                # first two score tiles right away so Scalar's exp pipeline
                # starts before the v-chain and transposes
                s_cur = emit_s(0)
                s_nxt0 = emit_s(1) if nkt > 1 else None
                emit_chain("v")
                for kt in range(4 * qc, 4 * qc + 4):
                    ps = psmm.tile([128, 128], BF16, tag="mm", name=f"pst{b}_{kt}")
                    nc.tensor.transpose(
                        ps[:], vT[:, b * S + kt * 128: b * S + (kt + 1) * 128],
                        ident_sb,
                    )
                    vn = vnatp.tile([128, 130], BF16, tag=f"vn{b}_{kt}", name=f"vn{b}_{kt}")
                    nc.vector.tensor_copy(
                        vn[:, 0:130].rearrange("p (g c) -> p g c", g=2, c=65)[:, :, 0:64],
                        ps[:].rearrange("p (g c) -> p g c", g=2, c=64),
                    )
                    nc.vector.memset(
                        vn[:, 0:130].rearrange("p (g c) -> p g c", g=2, c=65)[:, :, 64:65],
                        1.0,
                    )
                    v_nat[b][kt] = vn
                last_chunk = rc == 7
                if last_chunk:
                    upk = 1
                    unit_start = max(0, nkt - len(unit_q))
                else:
                    upk = 1 if unit_q else 0  # flat pacing; leftovers roll over
                    unit_start = 0
                scalar_ok = nkt <= 8  # scalar helps copies in low-exp chunks

                for kt in range(nkt):
                    s_ps, lo, d = s_cur
                    if kt + 2 < nkt:
                        s_nxt = s_nxt0
                        s_nxt0 = emit_s(kt + 2)
                    elif kt + 1 < nkt:
                        s_nxt = s_nxt0
                        s_nxt0 = None
                    else:
                        s_nxt = None
                    p_sb = work.tile([128, 1024], BF16, tag="p", name=f"p_{b}_{qc}_{kt}")
                    nc.scalar.activation(
                        p_sb[:, lo:1024], s_ps[:, lo:1024], EXP, scale=SCALE,
                    )
                    if d >= 0:
                        hi = min(512, d + 128)
                        for h in (0, 1):
                            nc.vector.tensor_mul(
                                p_sb[:, 512 * h + lo:512 * h + hi],
                                p_sb[:, 512 * h + lo:512 * h + hi],
                                mask_sb[:, 0:hi - lo],
                            )
                    for h in (0, 1):
                        nc.tensor.matmul(
                            o_ps[h][:, lo:512],
                            v_nat[b][kt][:, 65 * h:65 * h + 65],
                            p_sb[:, 512 * h + lo:512 * h + 512],
                            start=(kt == 0), stop=(kt == nkt - 1),
                        )
                    if kt >= unit_start:
                        for _ in range(min(upk, len(unit_q))):
                            emit_unit(scalar_ok=scalar_ok)
                    s_cur = s_nxt
                # normalize (per-head denominator on psum row 64) + stage
                ot = stagep.tile([128, 512], BF16, tag="ot", name=f"ot{b}_{qc}")
                for h in (0, 1):
                    rc_sb = work.tile([128, 512], F32, tag="recip", name=f"rc{b}_{qc}_{h}")
                    nc.vector.tensor_copy(rc_sb[0:1, :], o_ps[h][64:65, :])
                    nc.vector.reciprocal_approx_fast(rc_sb[0:1, :], rc_sb[0:1, :])
                    nc.gpsimd.partition_broadcast(
                        rc_sb[0:64, :], rc_sb[0:1, :], channels=64
                    )
                    nc.vector.tensor_mul(
                        ot[64 * h:64 * h + 64, :], o_ps[h][0:64, :], rc_sb[0:64, :]
                    )
                queue_outproj(b, qc, ot)

            while unit_q:
                emit_unit(scalar_ok=True, fine_dma=True)

    nc.compile()
    return nc


def kernel(x, Wq, bq, Wk, bk, Wv, bv, Wo):
    if "nc" not in _cache:
        _cache["nc"] = _build()
    nc = _cache["nc"]

    bf = ml_dtypes.bfloat16
    xt = np.ascontiguousarray(np.asarray(x, np.float32).reshape(B * S, DM).T).astype(bf)
    wo_f = np.asarray(Wo, np.float32)
    trimask = np.triu(np.ones((128, 128), np.float32))
    ident = np.eye(128, dtype=np.float32)

    in_maps = []
    for c in range(N_CORES):
        sl = slice(c * FPC, (c + 1) * FPC)
        wpk = np.empty((128, 3 * 8 * 128 + DM + 256), np.float32)
        for pr, W in enumerate((Wq, Wk, Wv)):
            Wc = np.asarray(W, np.float32)[:, sl]          # [1024, 128]
            wpk[:, pr * 1024:(pr + 1) * 1024] = (
                Wc.reshape(8, 128, 128).transpose(1, 0, 2).reshape(128, 1024)
            )
        wpk[:, 3072:3072 + DM] = wo_f[sl, :]
        wpk[:, 3072 + DM:3072 + DM + 128] = trimask
        wpk[:, 3072 + DM + 128:] = ident
        bpk = np.stack(
            [np.asarray(b, np.float32)[sl] for b in (bq, bk, bv)], axis=1
        )
        in_maps.append({
            "xt": xt,
            "wpk": np.ascontiguousarray(wpk).astype(bf),
            "bpk": np.ascontiguousarray(bpk),
        })

    trace = bool(int(os.environ.get("ATTN_KERNEL_TRACE", "0")))
    res = run_bass_kernel_spmd(nc, in_maps, core_ids=list(range(N_CORES)), trace=trace)
    if trace:
        print(f"HW exec time: {res.exec_time_ns} ns")
        _cache["exec_time_ns"] = res.exec_time_ns

    out = np.asarray(res.results[0]["out"]).astype(np.float32)
    for c in range(1, N_CORES):
        out += np.asarray(res.results[c]["out"]).astype(np.float32)
    return out
